# revision 38
# baseline (speedup 1.0000x reference)
"""Trainium2 Bass kernel for nn_Attention_57166014709861.

8-batch image attention (B=8, C=384, h=8, d=48, HW=1024), data-parallel:
one batch image per NeuronCore, weights broadcast, host-side gather.
HW exec ~143us on a TRN2 chip (baseline serial implementation: 222us).

Per-core pipeline (qkv/sT/proj matmuls in float32r = PE fp32 HIGH mode,
av in bf16; inputs pre-laid-out host-side to [partition, chunk, free]):
  qkv:   q,k packed [d,seq] head-pair tiles (2 heads per 128 partitions at
         offsets 0/64, scale folded into wq); v computed transposed as
         vT [seq, packed-c] with a leading ones column per head so the
         softmax denominator rides the av matmul for free (psum row 0).
  attn:  per head pair, interleaved through the y-tile loop (amortizes the
         ACT<->PE handoff; ACT exp and PE matmuls are co-bottlenecks):
         sT[y,x] = k_h^T q_h -> exp on ACT (no max subtraction; |s|<~6)
         -> av accumulates u'[c,x] + denominator over y tiles in PSUM.
         Software-pipelined: sT(yt+1) issues before av(yt) so the in-order
         PE never waits on exp.
  norm:  denominator row is DMA-transposed [1,1024]->[128,8] via a DRAM
         bounce (DVE reciprocal is per-lane serial: 6.5us on 1 partition,
         0.2us on 128), reciprocal in bf16, broadcast back with a stride-0
         DMA read, one DVE multiply per head. Runs entirely off the PE,
         two heads pipelined behind the next pair's compute.
  proj:  w_projT packed on K with zero pad rows; kt0-2 of all three output
         tiles run while the last pair's epilogue drains, kt3 lands last;
         bias added on ACT during psum->sbuf copy, DMA out.
"""

import sys

if "/opt/trn_rl_repo" not in sys.path:
    sys.path.insert(0, "/opt/trn_rl_repo")

import numpy as np

import concourse.bass as bass
import concourse.mybir as mybir
import concourse.tile as tile
from concourse import bacc
from concourse.bass_utils import run_bass_kernel_spmd

DIM = 384
HEADS = 8
DH = 48
SEQ = 1024
P = 128
NCORES = 8
VP = 64  # packed v cols per head: ones at col 0, zeros 1-15, 48 data at 16-63

F32 = mybir.dt.float32
F32R = mybir.dt.float32r
BF16 = mybir.dt.bfloat16
EXP = mybir.ActivationFunctionType.Exp
ADD = mybir.AluOpType.add
MULT = mybir.AluOpType.mult

_NC_CACHE = {}


def _emit(tc, nc, x_d, wq_d, wk_d, wv_d, wp_d, b_d, out_d):
    with (
        tc.tile_pool(name="const", bufs=1) as constp,
        tc.tile_pool(name="weights", bufs=1) as wpool,
        tc.tile_pool(name="data", bufs=1) as data,
        tc.tile_pool(name="ptile", bufs=6) as ppool,
        tc.tile_pool(name="bcpool", bufs=3) as bcpool,
        tc.tile_pool(name="rpool", bufs=3) as rpool,
        tc.tile_pool(name="avcp", bufs=3) as avcp,
        tc.tile_pool(name="opool", bufs=2) as opool,
        tc.tile_pool(name="ps_s", bufs=2, space="PSUM") as ps_s,
        tc.tile_pool(name="ps_av", bufs=2, space="PSUM") as ps_av,
        tc.tile_pool(name="dram", bufs=3, space="DRAM") as drampool,
    ):
        # ---- loads (ordered so the first qkv matmuls can start early) ----
        x_sb = data.tile([P, 3, SEQ], F32R, tag="x")
        wq_sb = wpool.tile([P, 3, 512], F32R, tag="wq")
        wk_sb = wpool.tile([P, 3, 512], F32R, tag="wk")
        # inputs are pre-laid-out host-side; per-ko interleave on three issue
        # queues so the first q-chain step (x0, wq0) can start ~4us in
        qs = [nc.sync, nc.scalar, nc.gpsimd]
        for ko in range(3):
            qs[ko % 3].dma_start(x_sb[:, ko, :], x_d.ap()[:, ko, :])
            qs[(ko + 1) % 3].dma_start(wq_sb[:, ko, :], wq_d.ap()[:, ko, :])
            qs[(ko + 2) % 3].dma_start(wk_sb[:, ko, :], wk_d.ap()[:, ko, :])
        wv_sb = wpool.tile([P, 3, HEADS * VP], F32R, tag="wv")
        nc.gpsimd.dma_start(wv_sb[:], wv_d.ap())
        wp_sb = wpool.tile([P, 4, DIM], F32R, tag="wp")
        nc.gpsimd.dma_start(wp_sb[:], wp_d.ap())
        bias_sb = constp.tile([P, 3], F32, tag="bias")
        nc.sync.dma_start(bias_sb[:], b_d.ap())
        zb_sb = constp.tile([P, 1], F32, tag="zb")
        nc.gpsimd.memset(zb_sb[:], 0.0)
        ones_sb = constp.tile([1, 64], BF16, tag="ones")
        nc.gpsimd.memset(ones_sb[:], 1.0)

        # ---- qkv ----
        q_sb = data.tile([P, 4, SEQ], F32R, tag="q")
        k_sb = data.tile([P, 4, SEQ], F32R, tag="k")
        vT_sb = data.tile([P, 8, HEADS, VP], BF16, tag="vT")

        for dst, w in ((q_sb, wq_sb), (k_sb, wk_sb)):
            for t in range(4):
                ps = ps_s.tile([P, SEQ], F32, tag="s", name="qk_ps")
                for j in range(2):
                    for ko in range(3):
                        nc.tensor.matmul(
                            ps[:, j * 512 : (j + 1) * 512],
                            lhsT=w[:, ko, t * 128 : (t + 1) * 128],
                            rhs=x_sb[:, ko, j * 512 : (j + 1) * 512],
                            start=(ko == 0),
                            stop=(ko == 2),
                        )
                nc.vector.tensor_copy(dst[:, t, :], ps[:])

        for yt in range(8):
            ps = ps_av.tile([P, SEQ], F32, tag="av", name="v_ps")
            for ko in range(3):
                nc.tensor.matmul(
                    ps[:, 0 : HEADS * VP],
                    lhsT=x_sb[:, ko, yt * 128 : (yt + 1) * 128],
                    rhs=wv_sb[:, ko, :],
                    start=(ko == 0),
                    stop=(ko == 2),
                )
            nc.vector.tensor_copy(
                vT_sb[:, yt, :, :],
                ps[:, 0 : HEADS * VP].rearrange("p (h v) -> p h v", h=HEADS),
            )
            # ones column (col 0) for the softmax denominator (psum row 0)
            nc.gpsimd.memset(vT_sb[:, yt, :, 0:1], 1.0)

        # ---- attention ----
        u_sb = [data.tile([P, SEQ], F32R, tag=f"u{i}", name=f"u{i}") for i in range(4)]

        def epi_a(h, av_copy, q=None):
            # av_copy row 0 = softmax denominator. DMA-transpose to [128, 8]
            # via a DRAM bounce so the reciprocal runs on 128 lanes.
            q = q or nc.sync
            den_dram = drampool.tile([SEQ], F32, tag="den")
            q.dma_start(den_dram[:], av_copy[0:1, :])
            den_pm = rpool.tile([P, 8], F32, tag="denpm")
            q.dma_start(den_pm[:], den_dram[:].rearrange("(p f) -> p f", p=P))
            rec_pm = rpool.tile([P, 8], BF16, tag="recpm")
            with nc.allow_low_precision(reason="softmax denom reciprocal to bf16"):
                nc.vector.reciprocal(rec_pm[:], den_pm[:])
            rec_dram = drampool.tile([SEQ], BF16, tag="rec")
            q.dma_start(rec_dram[:], rec_pm[:])
            return rec_dram

        def epi_b(h, av_copy, rec_dram, q=None):
            # broadcast 1/denom across 64 partitions with a stride-0 DMA read
            q = q or nc.sync
            t, s = h // 2, h % 2
            po = s * 64
            bc_sb = bcpool.tile([64, SEQ], BF16, tag="bcs")
            q.dma_start(
                bc_sb[:],
                rec_dram[:].rearrange("(o f) -> o f", o=1).to_broadcast([64, SEQ]),
            )
            nc.vector.tensor_tensor(
                u_sb[t][po : po + 64, :], av_copy[:], bc_sb[:], MULT
            )

        # Software-pipelined and head-pair interleaved: per yt the PE issues
        # sT for both heads of the pair, then av(yt-1) for both; the ACT
        # handoff amortizes over two exps.
        pending = []
        for t in range(4):
            avs = [
                ps_av.tile([P, SEQ], F32, tag="av", name=f"av{t}_{s}")
                for s in range(2)
            ]
            p_tiles = [[None] * 8 for _ in range(2)]
            for yt in range(9):
                for s in range(2):
                    po = s * 64
                    h = 2 * t + s
                    if yt < 8:
                        sT_ps = ps_s.tile([P, SEQ], F32, tag="s", name="sT_ps")
                        for j in range(2):
                            nc.tensor.matmul(
                                sT_ps[:, j * 512 : (j + 1) * 512],
                                lhsT=k_sb[
                                    po : po + 48, t, yt * 128 : (yt + 1) * 128
                                ],
                                rhs=q_sb[po : po + 48, t, j * 512 : (j + 1) * 512],
                                start=True,
                                stop=True,
                            )
                        p_tiles[s][yt] = ppool.tile(
                            [P, SEQ], BF16, tag="p", name="p_sb"
                        )
                        nc.scalar.activation(
                            p_tiles[s][yt][:], sT_ps[:], EXP, bias=zb_sb[:]
                        )
                for s in range(2):
                    h = 2 * t + s
                    if yt > 0:
                        for j in range(2):
                            nc.tensor.matmul(
                                avs[s][0:VP, j * 512 : (j + 1) * 512],
                                lhsT=vT_sb[:, yt - 1, h, :],
                                rhs=p_tiles[s][yt - 1][:, j * 512 : (j + 1) * 512],
                                start=(yt == 1),
                                stop=(yt == 8),
                            )
                if yt in (1, 2) and len(pending) >= yt and len(pending[yt - 1]) == 2:
                    pending[yt - 1] = (*pending[yt - 1], epi_a(*pending[yt - 1]))
                if yt in (5, 6) and len(pending) >= yt - 4:
                    if pending[yt - 5] is not None and len(pending[yt - 5]) == 3:
                        epi_b(*pending[yt - 5])
                        pending[yt - 5] = None
            pending = [p for p in pending if p is not None]
            for s in range(2):
                h = 2 * t + s
                av_copy = avcp.tile([64, SEQ], F32, tag="avc", name=f"avc{h}")
                nc.vector.tensor_copy(av_copy[:], avs[s][0:64, :])
                pending.append((h, av_copy))
        for i, pend in enumerate(pending):
            q = nc.scalar if i % 2 else nc.sync
            rec = epi_a(pend[0], pend[1], q)
            epi_b(pend[0], pend[1], rec, q)

        # ---- proj ----
        # kt3 (heads 6/7) lands last; run kt0-2 of all three output tiles
        # first so the PE stays busy while the final head's epilogue drains.
        pr_ps = []
        for ot in range(3):
            pool = ps_s if ot < 2 else ps_av
            ps = pool.tile(
                [P, SEQ], F32, tag=("s" if ot < 2 else "av"), name=f"prps{ot}"
            )
            pr_ps.append(ps)
            for j in range(2):
                for kt in range(3):
                    nc.tensor.matmul(
                        ps[:, j * 512 : (j + 1) * 512],
                        lhsT=wp_sb[:, kt, ot * 128 : (ot + 1) * 128],
                        rhs=u_sb[kt][:, j * 512 : (j + 1) * 512],
                        start=(kt == 0),
                        stop=False,
                    )
        # keep the PE p-state warm while the final epilogue chain drains
        # (results overwritten: same psum range is re-accumulated legitimately
        # below would be wrong — so burn into a scratch tile instead)
        warm_ps = ps_s.tile([P, SEQ], F32, tag="s", name="warm")
        for r in range(6):
            nc.tensor.matmul(
                warm_ps[:, 0:512],
                lhsT=wp_sb[:, 0, 0:128],
                rhs=u_sb[0][:, 0:512],
                start=True,
                stop=True,
            )
        for ot in range(3):
            o_sb = opool.tile([P, SEQ], F32, tag="o")
            for j in range(2):
                nc.tensor.matmul(
                    pr_ps[ot][:, j * 512 : (j + 1) * 512],
                    lhsT=wp_sb[:, 3, ot * 128 : (ot + 1) * 128],
                    rhs=u_sb[3][:, j * 512 : (j + 1) * 512],
                    start=False,
                    stop=True,
                )
            nc.scalar.activation(
                o_sb[:],
                pr_ps[ot][:],
                mybir.ActivationFunctionType.Identity,
                bias=bias_sb[:, ot : ot + 1],
            )
            nc.sync.dma_start(out_d.ap()[ot * 128 : (ot + 1) * 128, :], o_sb[:])


def build_nc():
    nc = bacc.Bacc("TRN2", target_bir_lowering=False, debug=False, num_devices=NCORES)
    x_d = nc.dram_tensor("x", [P, 3, SEQ], F32R, kind="ExternalInput")
    wq_d = nc.dram_tensor("wq", [P, 3, 512], F32R, kind="ExternalInput")
    wk_d = nc.dram_tensor("wk", [P, 3, 512], F32R, kind="ExternalInput")
    wv_d = nc.dram_tensor("wv", [P, 3, HEADS * VP], F32R, kind="ExternalInput")
    wp_d = nc.dram_tensor("wp", [P, 4, DIM], F32R, kind="ExternalInput")
    b_d = nc.dram_tensor("bias", [P, 3], F32, kind="ExternalInput")
    out_d = nc.dram_tensor("out", [DIM, SEQ], F32, kind="ExternalOutput")

    with tile.TileContext(nc) as tc:
        _emit(tc, nc, x_d, wq_d, wk_d, wv_d, wp_d, b_d, out_d)
    nc.compile()
    return nc


def pack_inputs(x, w_qkv, w_proj, b_proj):
    """Host-side weight packing. Returns per-core input maps."""
    x = np.asarray(x, np.float32)
    w_qkv = np.asarray(w_qkv, np.float32)
    w_proj = np.asarray(w_proj, np.float32)
    b_proj = np.asarray(b_proj, np.float32)
    scale = DH ** -0.5
    w_q, w_k, w_v = w_qkv[0:DIM], w_qkv[DIM : 2 * DIM], w_qkv[2 * DIM :]

    WQ = np.zeros((DIM, 512), np.float32)
    WK = np.zeros((DIM, 512), np.float32)
    WV = np.zeros((DIM, HEADS * VP), np.float32)
    WP = np.zeros((512, DIM), np.float32)
    for h in range(HEADS):
        col = (h // 2) * 128 + (h % 2) * 64
        WQ[:, col : col + DH] = (w_q[h * DH : (h + 1) * DH] * scale).T
        WK[:, col : col + DH] = w_k[h * DH : (h + 1) * DH].T
        WV[:, h * VP + 16 : h * VP + 16 + DH] = w_v[h * DH : (h + 1) * DH].T
        WP[col + 16 : col + 16 + DH, :] = w_proj[:, h * DH : (h + 1) * DH].T
    BIAS = np.ascontiguousarray(b_proj.reshape(3, P).T)

    def pm(a, chunks):
        # [(chunks*P), f] -> [P, chunks, f] partition-major pre-layout
        return np.ascontiguousarray(
            a.reshape(chunks, P, a.shape[-1]).transpose(1, 0, 2)
        )

    WQp, WKp, WVp, WPp = pm(WQ, 3), pm(WK, 3), pm(WV, 3), pm(WP, 4)
    in_maps = []
    for b in range(NCORES):
        in_maps.append(
            {
                "x": pm(x[b].reshape(DIM, SEQ), 3),
                "wq": WQp,
                "wk": WKp,
                "wv": WVp,
                "wp": WPp,
                "bias": BIAS,
            }
        )
    return in_maps


def run(in_maps, trace=False):
    if "nc" not in _NC_CACHE:
        _NC_CACHE["nc"] = build_nc()
    nc = _NC_CACHE["nc"]
    res = run_bass_kernel_spmd(
        nc, in_maps, core_ids=list(range(NCORES)), trace=trace
    )
    out = np.stack([res.results[i]["out"] for i in range(NCORES)])
    return out.reshape(NCORES, DIM, 32, 32), res


def kernel(x, w_qkv, w_proj, b_proj):
    out, _ = run(pack_inputs(x, w_qkv, w_proj, b_proj))
    return out
